# revision 1
# baseline (speedup 1.0000x reference)
"""Distributed GCN (2-layer + readout) on 8 Trainium2 NeuronCores.

Graph/data parallel per the sharding hint: nodes sharded 8-way by dst
owner (12500 real + pad -> 12544 local rows per core).

Per GCN layer:
- each core computes its table shard s = dinv * (h @ W) on TensorE,
- AllGather materializes the full 100352-row table in every core's HBM,
- per-edge messages stream via gpsimd.dma_gather (int16 indices -> the
  table is addressed as 4 chunks of 25088 rows),
- messages accumulate into the local agg buffer via gpsimd.dma_scatter_add.

dma_scatter_add loses updates for duplicate indices within one
instruction (RMW race across the 16 DMA engines), so edges are layered
by per-dst occurrence rank: each scatter instruction carries unique dst
indices (padding goes to scratch rows). Instructions on the same buffer
serialize (Tile WAW); two alternating accumulators (A/B) on separate
SWDGE queues keep descriptor generation overlapping the DMA drain.
Self-loops fold into the epilogue:
out = relu(dinv * (aggA + aggB + s_local) + b). The per-half AllGathers
let each half-table ship as soon as that half of the shard is computed.
"""
import numpy as np

from concourse import bass, bacc, tile, mybir, bass_utils

F32 = mybir.dt.float32
I16 = mybir.dt.int16

NCORES = 8
SCATTER_CAP = 4096
SCRATCH = 1024
D = 64


def _roundup(x, m=128):
    return (x + m - 1) // m * m


def preprocess(edge_index, n_nodes):
    """Build all index structures. Pure numpy, structural only."""
    src = np.asarray(edge_index[0], dtype=np.int64)
    dst = np.asarray(edge_index[1], dtype=np.int64)
    REAL = (n_nodes + NCORES - 1) // NCORES          # 12500
    NL = _roundup(REAL)                               # 12544
    CHUNK = NL * NCORES // 4                          # 25088
    assert CHUNK < 32768
    HALF = NL // 2                                    # 6272
    NT = NL * NCORES                                  # 100352

    owner = dst // REAL
    loc = dst % REAL
    # source position inside the per-half AllGather output:
    # half h = src_loc // HALF, row = owner*HALF + (src_loc % HALF);
    # each half-table [8*HALF rows] is addressed as 2 chunks of CHUNK.
    sloc = src % REAL
    s_h = sloc // HALF
    s_pos = (src // REAL) * HALF + (sloc % HALF)
    j_of = s_h * 2 + s_pos // CHUNK
    idx16 = s_pos % CHUNK

    deg = np.bincount(dst, minlength=n_nodes).astype(np.float32) + 1.0

    percore = []
    from collections import defaultdict
    cellcnt = defaultdict(lambda: np.zeros(NCORES, np.int64))
    maxl = 0
    for c in range(NCORES):
        m = owner == c
        l_c = loc[m]
        i16_c = idx16[m]
        order = np.argsort(l_c, kind="stable")
        lsort = l_c[order]
        occ = np.arange(len(lsort)) - np.searchsorted(lsort, lsort)
        occ_un = np.empty_like(occ)
        occ_un[order] = occ
        maxl = max(maxl, int(occ.max()) + 1 if len(occ) else 1)
        percore.append((l_c, i16_c, j_of[m], occ_un))
        key = occ_un * 4 + j_of[m]
        uk, cnt = np.unique(key, return_counts=True)
        for k, n in zip(uk, cnt):
            cellcnt[int(k)][c] = n

    cellsize = {k: _roundup(int(v.max())) for k, v in cellcnt.items()}

    # layout: for l, for j -> cell at running base
    cells = []   # (l, j, base, size)
    base = 0
    for ll in range(maxl):
        for j in range(4):
            k = ll * 4 + j
            if k in cellsize:
                cells.append((ll, j, base, cellsize[k]))
                base += cellsize[k]
    TOT = base

    # scatter instructions: pack cells of one l-block up to SCATTER_CAP
    sinstrs = []  # (base, size)
    i = 0
    while i < len(cells):
        ll, _, b0, sz = cells[i]
        size = sz
        i += 1
        while (i < len(cells) and cells[i][0] == ll
               and size + cells[i][3] <= SCATTER_CAP):
            size += cells[i][3]
            i += 1
        sinstrs.append((b0, size))

    gidx_all, sidx_all = [], []
    for c in range(NCORES):
        l_c, i16_c, j_c, occ_c = percore[c]
        gidx = np.zeros(TOT, np.int16)
        sdst = np.full(TOT, -1, np.int64)
        for (ll, j, b, sz) in cells:
            m = (occ_c == ll) & (j_c == j)
            n = int(m.sum())
            assert n <= sz
            gidx[b:b + n] = i16_c[m].astype(np.int16)
            sdst[b:b + n] = l_c[m]
        for (b, size) in sinstrs:
            seg = sdst[b:b + size]
            pad = seg == -1
            npad = int(pad.sum())
            assert npad <= SCRATCH, f"npad={npad}"
            seg[pad] = NL + np.arange(npad)
        gw = np.zeros((128, TOT // 16), np.int16)
        for (ll, j, b, sz) in cells:
            w = gidx[b:b + sz].reshape(-1, 16).T
            gw[:, b // 16:(b + sz) // 16] = np.tile(w, (8, 1))
        sw = np.zeros((128, TOT // 16), np.int16)
        for (b, size) in sinstrs:
            w = sdst[b:b + size].astype(np.int16).reshape(-1, 16).T
            sw[:, b // 16:(b + size) // 16] = np.tile(w, (8, 1))
        gidx_all.append(gw)
        sidx_all.append(sw)

    deg_tiles = []
    for c in range(NCORES):
        d = np.ones(NL, np.float32)
        lo, hi = c * REAL, min((c + 1) * REAL, n_nodes)
        d[:hi - lo] = deg[lo:hi]
        deg_tiles.append(np.ascontiguousarray(
            d.reshape(NL // 128, 128).T))          # [128, G]

    meta = dict(REAL=REAL, NL=NL, CHUNK=CHUNK, HALF=HALF, NT=NT, TOT=TOT,
                cells=cells, sinstrs=sinstrs, maxl=maxl)
    return meta, gidx_all, sidx_all, deg_tiles


def build(meta):
    NL, CHUNK, HALF, NT, TOT = (meta["NL"], meta["CHUNK"], meta["HALF"],
                                meta["NT"], meta["TOT"])
    cells, sinstrs = meta["cells"], meta["sinstrs"]
    G = NL // 128
    GH = G // 2
    AGGR = NL + SCRATCH

    nc = bacc.Bacc("TRN2", target_bir_lowering=False, debug=False,
                   num_devices=NCORES, num_swdge_queues=4)

    xT = nc.dram_tensor("xT", [D, NL], F32, kind="ExternalInput")
    W1 = nc.dram_tensor("W1", [D, D], F32, kind="ExternalInput")
    W2 = nc.dram_tensor("W2", [D, D], F32, kind="ExternalInput")
    b1e = nc.dram_tensor("b1bc", [128, D], F32, kind="ExternalInput")
    b2e = nc.dram_tensor("b2bc", [128, D], F32, kind="ExternalInput")
    woute = nc.dram_tensor("woutbc", [128, D], F32, kind="ExternalInput")
    boute = nc.dram_tensor("boutbc", [128, 1], F32, kind="ExternalInput")
    dege = nc.dram_tensor("deg", [128, G], F32, kind="ExternalInput")
    gidxe = nc.dram_tensor("gidx", [128, TOT // 16], I16, kind="ExternalInput")
    sidxe = nc.dram_tensor("sidx", [128, TOT // 16], I16, kind="ExternalInput")
    idente = nc.dram_tensor("ident", [128, 128], F32, kind="ExternalInput")
    oute = nc.dram_tensor("out", [128, G], F32, kind="ExternalOutput")

    ag_in = [[nc.dram_tensor(f"ag_in{L}_{h}", [HALF, D], F32)
              for h in (0, 1)] for L in (0, 1)]
    ag_out = [[nc.dram_tensor(f"ag_out{L}_{h}", [NCORES * HALF, D], F32,
                              addr_space="Shared")
               for h in (0, 1)] for L in (0, 1)]
    agg = [[nc.dram_tensor(f"agg{L}_{ab}", [AGGR, D], F32)
            for ab in range(2)] for L in (0, 1)]

    def shard_view(dram):
        # [NL, 64] dram viewed as [128, G, 64] with row r = g*128 + p
        return dram.ap().rearrange("(g p) d -> p g d", p=128)

    def half_view(dram, h):
        return dram.ap()[h * HALF:(h + 1) * HALF, :].rearrange(
            "(g p) d -> p g d", p=128)

    with tile.TileContext(nc) as tc:
        with (
            tc.tile_pool(name="pool", bufs=1) as pool,
            tc.tile_pool(name="xs", bufs=2) as xspool,
            tc.tile_pool(name="msg", bufs=5) as msgpool,
            tc.tile_pool(name="epi", bufs=1) as epipool,
            tc.tile_pool(name="psum", bufs=2, space="PSUM") as psum,
        ):
            gidx_t = pool.tile([128, TOT // 16], I16, tag="gidx")
            sidx_t = pool.tile([128, TOT // 16], I16, tag="sidx")
            nc.scalar.dma_start(out=gidx_t[:], in_=gidxe[:])
            nc.scalar.dma_start(out=sidx_t[:], in_=sidxe[:])
            W1_t = pool.tile([D, D], F32, tag="w1")
            W2_t = pool.tile([D, D], F32, tag="w2")
            nc.scalar.dma_start(out=W1_t[:], in_=W1[:])
            nc.scalar.dma_start(out=W2_t[:], in_=W2[:])
            b1_t = pool.tile([128, D], F32, tag="b1")
            b2_t = pool.tile([128, D], F32, tag="b2")
            wout_t = pool.tile([128, D], F32, tag="wout")
            bout_t = pool.tile([128, 1], F32, tag="bout")
            ident_t = pool.tile([128, 128], F32, tag="ident")
            nc.scalar.dma_start(out=b1_t[:], in_=b1e[:])
            nc.scalar.dma_start(out=b2_t[:], in_=b2e[:])
            nc.scalar.dma_start(out=wout_t[:], in_=woute[:])
            nc.scalar.dma_start(out=bout_t[:], in_=boute[:])
            nc.scalar.dma_start(out=ident_t[:], in_=idente[:])
            deg_t = pool.tile([128, G], F32, tag="deg")
            nc.sync.dma_start(out=deg_t[:], in_=dege[:])
            dinv_t = pool.tile([128, G], F32, tag="dinv")
            nc.scalar.activation(dinv_t[:], deg_t[:],
                                 mybir.ActivationFunctionType.Sqrt)
            nc.vector.reciprocal(dinv_t[:], dinv_t[:])

            # --- layer-1 table shard: s1 = dinv * (x @ W1) ---
            # built per half; each half's AllGather launches as soon as
            # that half of the shard is ready.
            s1_t = pool.tile([128, G, D], F32, tag="s1")
            for h in (0, 1):
                g0, g1 = h * GH, (h + 1) * GH
                for t0 in range(g0, g1, 8):
                    nt = min(8, g1 - t0)
                    xT_t = xspool.tile([D, 8 * 128], F32, tag="xT")
                    nc.sync.dma_start(out=xT_t[:, :nt * 128],
                                      in_=xT[:, t0 * 128:(t0 + nt) * 128])
                    pt = psum.tile([128, 512], F32, tag="mm")
                    for t in range(t0, t0 + nt):
                        nc.tensor.matmul(
                            pt[:, (t - t0) * D:(t - t0 + 1) * D],
                            xT_t[:, (t - t0) * 128:(t - t0 + 1) * 128],
                            W1_t[:])
                    for t in range(t0, t0 + nt):
                        nc.vector.tensor_scalar_mul(
                            s1_t[:, t, :],
                            pt[:, (t - t0) * D:(t - t0 + 1) * D],
                            dinv_t[:, t:t + 1])
                nc.sync.dma_start(out=shard_view(ag_in[0][h]),
                                  in_=s1_t[:, g0:g1, :])
                nc.gpsimd.collective_compute(
                    "AllGather", mybir.AluOpType.bypass,
                    replica_groups=[list(range(NCORES))],
                    ins=[ag_in[0][h].ap().opt()],
                    outs=[ag_out[0][h].ap().opt()])

            QF = AGGR * D // 128 // 4
            zeros_t = pool.tile([128, QF], F32, tag="zeros")
            nc.vector.memset(zeros_t[:], 0.0)
            for L in (0, 1):
                for ab in range(2):
                    zv = agg[L][ab].ap().rearrange(
                        "(p f) d -> p (f d)", p=128)
                    for piece in range(4):
                        nc.sync.dma_start(
                            out=zv[:, piece * QF:(piece + 1) * QF],
                            in_=zeros_t[:])

            def gs_layer(L):
                ab = 0
                gi = 0
                for (b, size) in sinstrs:
                    mt = msgpool.tile([128, size // 128, D], F32, tag="m")
                    for (cl, cj, cb, csz) in cells:
                        if cb < b or cb >= b + size:
                            continue
                        table = ag_out[L][cj // 2]
                        j2 = cj % 2
                        nc.gpsimd.dma_gather(
                            mt[:, (cb - b) // 128:(cb - b + csz) // 128, :],
                            table[j2 * CHUNK:(j2 + 1) * CHUNK, :],
                            gidx_t[:, cb // 16:(cb + csz) // 16],
                            num_idxs=csz, num_idxs_reg=csz, elem_size=D,
                            single_packet=False, queue_num=gi % 2)
                        gi += 1
                    nc.gpsimd.dma_scatter_add(
                        agg[L][ab][:], mt[:],
                        sidx_t[:, b // 16:(b + size) // 16],
                        num_idxs=size, num_idxs_reg=size, elem_size=D,
                        single_packet=False, queue_num=2 + ab)
                    ab ^= 1

            def epilogue_half(L, h, s_t, bias_t):
                """In place: s_t half h <- relu(dinv*(A+B+s) + b)."""
                av = s_t[:, h * GH:(h + 1) * GH, :]
                for ab in range(2):
                    ta = epipool.tile([128, GH, D], F32, tag=f"e{ab % 2}")
                    nc.sync.dma_start(out=ta[:], in_=half_view(agg[L][ab], h))
                    nc.vector.tensor_tensor(av, av, ta[:],
                                            mybir.AluOpType.add)
                dvb = dinv_t[:, h * GH:(h + 1) * GH].unsqueeze(
                    2).broadcast_to([128, GH, D])
                nc.vector.tensor_tensor(av, av, dvb, mybir.AluOpType.mult)
                bb = bias_t[:].unsqueeze(1).broadcast_to([128, GH, D])
                nc.vector.tensor_tensor(av, av, bb, mybir.AluOpType.add)
                nc.scalar.activation(av, av,
                                     mybir.ActivationFunctionType.Relu)

            gs_layer(0)

            # per half: epilogue 1, layer-2 table, half-AllGather
            s2_t = pool.tile([128, G, D], F32, tag="s2")
            for h in (0, 1):
                epilogue_half(0, h, s1_t, b1_t)   # a1 half in-place in s1_t
                for t in range(h * GH, (h + 1) * GH):
                    tp = psum.tile([64, 128], F32, tag="tr")
                    nc.tensor.transpose(tp[:], s1_t[:, t, :], ident_t[:])
                    a1T = pool.tile([64, 128], F32, tag="a1T")
                    nc.vector.tensor_copy(a1T[:], tp[:])
                    pt2 = psum.tile([128, D], F32, tag="mm2")
                    nc.tensor.matmul(pt2[:], a1T[:], W2_t[:])
                    nc.vector.tensor_scalar_mul(s2_t[:, t, :], pt2[:],
                                                dinv_t[:, t:t + 1])
                nc.sync.dma_start(out=shard_view(ag_in[1][h]),
                                  in_=s2_t[:, h * GH:(h + 1) * GH, :])
                nc.gpsimd.collective_compute(
                    "AllGather", mybir.AluOpType.bypass,
                    replica_groups=[list(range(NCORES))],
                    ins=[ag_in[1][h].ap().opt()],
                    outs=[ag_out[1][h].ap().opt()])

            gs_layer(1)
            for h in (0, 1):
                epilogue_half(1, h, s2_t, b2_t)   # a2 in-place in s2_t

            # --- readout: out = (a2 * WoutT).sum(d) + bout ---
            o_t = pool.tile([128, G], F32, tag="o")
            wb = wout_t[:].unsqueeze(1).broadcast_to([128, G, D])
            nc.vector.tensor_tensor(s2_t[:], s2_t[:], wb,
                                    mybir.AluOpType.mult)
            nc.vector.tensor_reduce(o_t[:], s2_t[:],
                                    axis=mybir.AxisListType.X,
                                    op=mybir.AluOpType.add)
            nc.vector.tensor_scalar_add(o_t[:], o_t[:], bout_t[:])
            nc.sync.dma_start(out=oute[:], in_=o_t[:])

    nc.compile()
    return nc


_CACHE = {}


def kernel(x, edge_index, batch, W1, b1, W2, b2, Wout, bout, _trace=False):
    x = np.asarray(x, np.float32)
    edge_index = np.asarray(edge_index)
    W1 = np.asarray(W1, np.float32)
    W2 = np.asarray(W2, np.float32)
    b1 = np.asarray(b1, np.float32)
    b2 = np.asarray(b2, np.float32)
    Wout = np.asarray(Wout, np.float32)
    bout = np.asarray(bout, np.float32).reshape(-1)
    N = x.shape[0]

    key = (N, edge_index.shape[1])
    if key not in _CACHE:
        meta, gidx_all, sidx_all, deg_tiles = preprocess(edge_index, N)
        nc = build(meta)
        _CACHE[key] = (meta, gidx_all, sidx_all, deg_tiles, nc)
    meta, gidx_all, sidx_all, deg_tiles, nc = _CACHE[key]
    REAL, NL = meta["REAL"], meta["NL"]

    ident = np.eye(128, dtype=np.float32)
    b1bc = np.tile(b1[None, :], (128, 1)).astype(np.float32)
    b2bc = np.tile(b2[None, :], (128, 1)).astype(np.float32)
    woutbc = np.tile(Wout.reshape(1, -1), (128, 1)).astype(np.float32)
    boutbc = np.full((128, 1), float(bout[0]), np.float32)

    in_maps = []
    for c in range(NCORES):
        xs = np.zeros((NL, D), np.float32)
        lo, hi = c * REAL, min((c + 1) * REAL, N)
        xs[:hi - lo] = x[lo:hi]
        in_maps.append({
            "xT": np.ascontiguousarray(xs.T),
            "W1": W1, "W2": W2, "b1bc": b1bc, "b2bc": b2bc,
            "woutbc": woutbc, "boutbc": boutbc,
            "deg": deg_tiles[c], "gidx": gidx_all[c], "sidx": sidx_all[c],
            "ident": ident,
        })

    res = bass_utils.run_bass_kernel_spmd(
        nc, in_maps, core_ids=list(range(NCORES)), trace=_trace)

    out = np.zeros(N, np.float32)
    for c in range(NCORES):
        o = res.results[c]["out"]
        arr = o.T.ravel()
        lo, hi = c * REAL, min((c + 1) * REAL, N)
        out[lo:hi] = arr[:hi - lo]
    if _trace:
        return out, res.exec_time_ns
    return out

